# revision 4
# baseline (speedup 1.0000x reference)
"""Multi-head attention (B=2, S=4096, D=512, H=8) on 8 Trainium2 NeuronCores.

Sharding: core c handles batch b = c // 4 and head-group g = c % 4 (2 heads =
columns/rows [128g : 128g+128] of the projection weights).  Each core runs its
2 heads' attention over the full sequence plus the partial output projection
through the matching 128 rows of Wo; the host sums the 4 partials per batch
(+ bo) in fp32.

The kernel is co-limited by the scalar (ACT) engine (exp over 2 x 4096 x
4096 logits at 1 elem/cycle/lane, ~262us) and the tensor engine (~310us),
so the schedule keeps ACT saturated and spreads all other PE work into the
attention pipeline's slack.  All DMAs keep a single xbar mode in flight at
a time (plain copies strictly before the transposes -- concurrent modes
serialize globally, and concurrent transposes on both HWDGE queues corrupt
data).  Inputs are DMA-transposed in pieces on the sync queue (k first --
it gates every chunk of the first iteration); q rows 0:512 are instead
plain-loaded and PE-transposed while the PE is idle at the head.  v is
projected directly in natural [keys, gd] orientation (its bias folds into
the host-side output constant since attention rows sum to 1).  Projection
matmuls, v chains, deferred PV (until v lands), and the output projection
are interleaved via per-chunk hooks.  Softmax denominators ride the
ones-column of vaug; each PV accumulator's denominator row is staged and
PE-transposed into per-partition reciprocals.  Only exp runs on ACT; the
output projection runs on PE + DVE and the partials are written in fp16.

Numerics: fp16 storage for X/W/q/k/v/P/ctx, fp32 PSUM accumulation, fp32
softmax denominators (no row-max: logits ~N(0,1), exp safe in fp32).
"""

import os
from collections import defaultdict

import numpy as np

import concourse.bass as bass
import concourse.tile as tile
from concourse import bacc, mybir
from concourse.bass_utils import run_bass_kernel_spmd
from concourse.masks import make_identity

P = 128
D = 512
GD = 128  # head-group width: 2 heads x 64
HD = 64
S_FULL = 4096
B_FULL = 2
N_CORES = 8
NT = S_FULL // P  # 32 key tiles
QB = S_FULL // 512  # 8 query blocks
CH = 3  # key-tiles per exp chunk (3 PSUM banks, double buffered)
NCH = (NT + CH - 1) // CH  # 11 chunks per (qb, h) iteration
PC = 1024  # piece height for input DMA transposes
NP = S_FULL // PC  # 4 pieces per tensor
F32 = mybir.dt.float32
F16 = mybir.dt.float16
I16 = mybir.dt.int16
EXP = mybir.ActivationFunctionType.Exp
MULT = mybir.AluOpType.mult
ADD = mybir.AluOpType.add

# Schraudolph fp16 exp on DVE: i16 = rint(lg * SCH_A + SCH_B) bitcast to fp16
# approximates exp(0.125 * lg) with relative error in [-3.01%, +3.02%]
# (DVE fp32->int16 conversion is round-to-nearest; HW-probed).  Routing a
# fraction of exp chunks here unloads the ACT engine.
SCH_A = 0.125 * 1024.0 * 1.4426950408889634
SCH_B = 1024.0 * (15.0 - 0.0436)
DVE_CHUNKS = (4, 9)  # chunk indices per iteration routed to DVE


def _emit(tc, io, dbg=None):
    nc = tc.nc
    xq, xk, xv, wq, wk, wv, wo, bq, bk, out = io

    with (
        tc.tile_pool(name="persist", bufs=1) as pp,
        tc.tile_pool(name="lgp", bufs=2, space="PSUM") as lgp,
        tc.tile_pool(name="mpsum", bufs=1, space="PSUM") as mp,
        tc.tile_pool(name="pbp", bufs=1, space="PSUM") as pbp,
        tc.tile_pool(name="xtp", bufs=16) as xtp,
        tc.tile_pool(name="q0p", bufs=4) as q0p,
        tc.tile_pool(name="ptp", bufs=20) as ptp,
        tc.tile_pool(name="stp", bufs=3) as stp,
        tc.tile_pool(name="obp", bufs=5) as obp,
    ):
        ident16 = pp.tile([P, P], F16, name="ident16")
        make_identity(nc, ident16)
        # fp16 weights (pre-cast on host)
        wqs = pp.tile([P, 4, GD], F16, name="wqs")
        wks = pp.tile([P, 4, GD], F16, name="wks")
        wvs = pp.tile([P, 4, GD], F16, name="wvs")
        nc.gpsimd.dma_start(wqs, wq.rearrange("(t p) m -> p t m", p=P))
        nc.gpsimd.dma_start(wks, wk.rearrange("(t p) m -> p t m", p=P))
        nc.gpsimd.dma_start(wvs, wv.rearrange("(t p) m -> p t m", p=P))
        wos = pp.tile([P, D], F16, name="wos")
        nc.gpsimd.dma_start(wos, wo)
        bqs = pp.tile([P, 1], F32, name="bqs")
        bks = pp.tile([P, 1], F32, name="bks")
        nc.gpsimd.dma_start(bqs, bq[:, None])
        nc.gpsimd.dma_start(bks, bk[:, None])

        # big persistent activations (all fp16)
        kT = pp.tile([P, S_FULL], F16, name="kT")
        qT0 = pp.tile([P, S_FULL], F16, name="qT0")
        qT1 = pp.tile([P, S_FULL], F16, name="qT1")
        qTh = [qT0, qT1]
        nc.gpsimd.memset(qT0[HD:P, :], 0.0)
        nc.gpsimd.memset(qT1[0:HD, :], 0.0)
        vaug0 = pp.tile([P, NT, P], F16, name="vaug0")
        vaug1 = pp.tile([P, NT, P], F16, name="vaug1")
        vaug = [vaug0, vaug1]
        nc.gpsimd.memset(vaug0, 0.0)
        nc.gpsimd.memset(vaug0[:, :, HD : HD + 1], 1.0)
        nc.gpsimd.memset(vaug1, 0.0)
        nc.gpsimd.memset(vaug1[:, :, 0:1], 1.0)
        uctx16 = pp.tile([P, S_FULL], F16, name="uctx16")
        # per-partition reciprocal denominators: rd[:, st, h]
        rd = pp.tile([P, NT, 2], F32, name="rd")

        # q rows 0:512 skip the serial transpose queue: plain loads (before
        # any transpose-mode DMA -- the xbar serializes mode transitions
        # globally) + PE transposes while the PE is idle at the head.
        xq0n = []
        for qt in range(4):
            t = q0p.tile([P, 512], F16, tag="xqn", name="xqn")
            nc.gpsimd.dma_start(t, xq[qt * P : (qt + 1) * P, :])
            xq0n.append(t)

        # ---------------- input DMA transposes, 1024-row pieces -------------
        dmaq = [nc.sync, nc.sync]
        xts = {}

        def emit_piece_dma(which, row0, nrows):
            src = {"k": xk, "v": xv, "q": xq}[which]
            tiles = []
            for dt in range(4):
                xt = xtp.tile([P, PC], F16, tag="xt", name="xt")[:, :nrows]
                dmaq[0].dma_start(
                    xt,
                    src[row0 : row0 + nrows, dt * P : (dt + 1) * P],
                    transpose=True,
                )
                tiles.append(xt)
            xts[(which, row0)] = tiles

        dma_order = [
            ("k", 0, 512), ("k", 512, 512),
            ("k", 1024, 1024), ("k", 2048, 1024), ("k", 3072, 1024),
            ("q", 512, 512),
            ("v", 0, 1024), ("v", 1024, 1024), ("v", 2048, 1024),
            ("v", 3072, 1024),
            ("q", 1024, 1024), ("q", 2048, 1024), ("q", 3072, 1024),
        ]
        for which, row0, nrows in dma_order:
            emit_piece_dma(which, row0, nrows)

        q0tiles = [
            q0p.tile([P, 512], F16, tag="xtq", name="xtq") for _ in range(4)
        ]
        for qt in range(4):
            for dt in range(4):
                pl, tg = (mp, "mA") if dt % 2 == 0 else (pbp, "pb")
                psq = pl.tile([P, P], F16, tag=tg, name="psq")
                nc.tensor.transpose(
                    psq, xq0n[qt][:, dt * P : (dt + 1) * P], ident16
                )
                nc.vector.tensor_copy(
                    out=q0tiles[dt][:, qt * P : (qt + 1) * P], in_=psq
                )
        xts[("q", 0)] = q0tiles

        # ------------- projections (emitted piecewise via hooks) ------------
        def emit_kq_proj(which, row0, nrows):
            w = {"k": wks, "q": wqs}[which]
            tiles = xts.pop((which, row0))
            for sbl in range(nrows // 512):
                cols = slice(row0 + sbl * 512, row0 + (sbl + 1) * 512)
                lcol = slice(sbl * 512, (sbl + 1) * 512)
                acc = mp.tile([P, 512], F32, tag="mA", name="acc")
                for dt in range(4):
                    nc.tensor.matmul(
                        acc,
                        lhsT=w[:, dt, :],
                        rhs=tiles[dt][:, lcol],
                        start=(dt == 0),
                        stop=(dt == 3),
                    )
                if which == "k":
                    nc.vector.tensor_scalar_add(kT[:, cols], acc[:], bks[:])
                else:
                    nc.vector.tensor_scalar_add(
                        qT0[0:HD, cols], acc[0:HD, :], bqs[0:HD, :]
                    )
                    nc.vector.tensor_scalar_add(
                        qT1[HD:P, cols], acc[HD:P, :], bqs[HD:P, :]
                    )

        def emit_v_chain(row0, nrows, g0=None):
            # v projected straight into natural [keys, gd] tiles:
            # psv[:, i, :] = 1 x bv  +  sum_dt xvT[dt,:].T @ Wv[dt,:]
            if g0 is None:
                tiles = xts.pop(("v", row0))
            else:
                tiles = xts[("v", row0)] if g0 == 0 else xts.pop(("v", row0))
            ktiles = nrows // P
            groups = range(ktiles // 4) if g0 is None else [g0]
            for g in groups:
                psv = mp.tile([P, 4, P], F32, tag="mA", name="psv")
                for i in range(4):
                    kl = 4 * g + i
                    lcol = slice(kl * P, (kl + 1) * P)
                    for dt in range(4):
                        nc.tensor.matmul(
                            psv[:, i, :],
                            lhsT=tiles[dt][:, lcol],
                            rhs=wvs[:, dt, :],
                            start=(dt == 0),
                            stop=(dt == 3),
                        )
                kt0 = row0 // P + 4 * g
                nc.vector.tensor_copy(
                    out=vaug0[:, kt0 : kt0 + 4, 0:HD], in_=psv[:, :, 0:HD]
                )
                nc.vector.tensor_copy(
                    out=vaug1[:, kt0 : kt0 + 4, HD:P], in_=psv[:, :, HD:P]
                )

        # ------------------ attention chunk pipeline ------------------------
        iters = [(qb, h) for qb in range(QB) for h in (0, 1)]
        total = len(iters) * NCH
        lg_tiles = {}
        ptt_tiles = {}
        pv_tiles = {}
        stage_tiles = {}

        def chunk_info(j):
            it_idx, ci = divmod(j, NCH)
            qb, h = iters[it_idx]
            c0 = ci * CH
            n = min(CH, NT - c0)
            return it_idx, qb, h, c0, n

        def emit_qk(j):
            it_idx, qb, h, c0, n = chunk_info(j)
            qcols = slice(qb * 512, (qb + 1) * 512)
            lg = lgp.tile([P, CH * 512], F32, tag="lg", name="lg")
            for i in range(n):
                kt_i = c0 + i
                nc.tensor.matmul(
                    lg[:, i * 512 : (i + 1) * 512],
                    lhsT=kT[:, kt_i * P : (kt_i + 1) * P],
                    rhs=qTh[h][:, qcols],
                    start=True,
                    stop=True,
                )
            lg_tiles[j] = lg

        def emit_pv(j):
            it_idx, qb, h, c0, n = chunk_info(j)
            if it_idx not in pv_tiles:
                pv_tiles[it_idx] = pbp.tile([P, 512], F32, tag="pb", name="pv")
            pv_acc = pv_tiles[it_idx]
            ptt = ptt_tiles.pop(j)
            for i in range(n):
                kt_i = c0 + i
                nc.tensor.matmul(
                    pv_acc,
                    lhsT=vaug[h][:, kt_i, :],
                    rhs=ptt[:, i * 512 : (i + 1) * 512],
                    start=(kt_i == 0),
                    stop=(kt_i == NT - 1),
                )

        def emit_tail(it_idx):
            qb, h = iters[it_idx]
            qcols = slice(qb * 512, (qb + 1) * 512)
            pv_acc = pv_tiles.pop(it_idx)
            rows = slice(0, HD) if h == 0 else slice(HD, P)
            drow = HD if h == 0 else 0
            if h == 0:
                stage_tiles[qb] = stp.tile([48, 512], F16, tag="stg", name="stg")
            stg = stage_tiles[qb]
            srow = 32 * h  # DVE partition offsets must be 32-aligned
            nc.vector.tensor_copy(out=uctx16[rows, qcols], in_=pv_acc[rows, :])
            nc.vector.tensor_copy(
                out=stg[srow : srow + 1, :], in_=pv_acc[drow : drow + 1, :]
            )

        # --------- output projection ops (one hook slot each) ---------------
        # Denominator rows -> per-partition columns via small PE transposes
        # (rows 1:32 of the slab are junk, landing in unread out columns),
        # then reciprocals on DVE.  heads=None does both heads at once.
        def c_transpose(qb, heads=None, pop=True):
            def fn():
                stg = stage_tiles.pop(qb) if pop else stage_tiles[qb]
                for sl in range(4):
                    st = 4 * qb + sl
                    tps = mp.tile([P, 33], F16, tag="mA", name="tps")
                    if heads is None:
                        nc.tensor.transpose(
                            tps,
                            stg[0:33, sl * P : (sl + 1) * P],
                            ident16[0:33, 0:33],
                        )
                        nc.vector.reciprocal(rd[:, st, 0:1], tps[:, 0:1])
                        nc.vector.reciprocal(rd[:, st, 1:2], tps[:, 32:33])
                    else:
                        (h,) = heads
                        nc.tensor.transpose(
                            tps[:, 0:1],
                            stg[32 * h : 32 * h + 1, sl * P : (sl + 1) * P],
                            ident16[32 * h : 32 * h + 1, 32 * h : 32 * h + 1],
                        )
                        nc.vector.reciprocal(
                            rd[:, st, h : h + 1], tps[:, 0:1]
                        )
            return fn

        ob0_tiles = {}

        def c_mm(qb, sl, half, pool=None, tag=None):
            def fn():
                st = 4 * qb + sl
                stcols = slice(st * P, (st + 1) * P)
                pl, tg = (pool or mp), (tag or "mA")
                ps = pl.tile([P, D], F32, tag=tg, name="cps")
                rows = slice(0, HD) if half == 0 else slice(HD, P)
                nc.tensor.matmul(
                    ps,
                    lhsT=uctx16[rows, stcols],
                    rhs=wos[rows, :],
                    start=True,
                    stop=True,
                )
                if half == 0:
                    ob0 = obp.tile([P, D], F32, tag="ob0", name="ob0")
                    nc.vector.tensor_scalar_mul(ob0, ps[:], rd[:, st, 0:1])
                    ob0_tiles[st] = ob0
                else:
                    ob = obp.tile([P, D], F16, tag="ob", name="ob")
                    nc.vector.scalar_tensor_tensor(
                        out=ob,
                        in0=ps[:],
                        scalar=rd[:, st, 1:2],
                        in1=ob0_tiles.pop(st),
                        op0=MULT,
                        op1=ADD,
                    )
                    nc.sync.dma_start(out[st * P : (st + 1) * P, :], ob)
            return fn

        def c_ops(qb, pool2=None, tag2=None):
            ops = [c_transpose(qb)]
            for sl in range(4):
                ops.append(c_mm(qb, sl, 0))
                ops.append(c_mm(qb, sl, 1, pool2, tag2))
            return ops

        # ----------------------- hook schedule ------------------------------
        hooks = defaultdict(list)

        def at(it_idx, ci, fn):
            hooks[(it_idx, ci)].append(fn)

        at(0, 0, lambda: emit_kq_proj("k", 512, 512))
        at(0, 1, lambda: emit_kq_proj("k", 1024, 1024))
        at(0, 4, lambda: emit_kq_proj("k", 2048, 1024))
        at(0, 7, lambda: emit_kq_proj("k", 3072, 1024))
        at(1, 2, lambda: emit_kq_proj("q", 512, 512))
        at(1, 3, lambda: emit_v_chain(0, 1024, 0))
        at(1, 4, lambda: emit_v_chain(0, 1024, 1))
        at(1, 7, lambda: emit_v_chain(1024, 1024, 0))
        at(1, 8, lambda: emit_v_chain(1024, 1024, 1))
        at(2, 0, lambda: emit_v_chain(2048, 1024, 0))
        at(2, 1, lambda: emit_v_chain(2048, 1024, 1))
        at(2, 3, lambda: emit_v_chain(3072, 1024, 0))
        at(2, 4, lambda: emit_v_chain(3072, 1024, 1))
        at(3, 6, lambda: emit_kq_proj("q", 1024, 1024))
        at(5, 0, lambda: emit_kq_proj("q", 2048, 1024))
        at(7, 0, lambda: emit_kq_proj("q", 3072, 1024))

        def pv_group(j0, j1):
            def fn():
                for j in range(j0, j1):
                    emit_pv(j)
            return fn

        # deferred PV + tails for iters 0..3 (until v pieces land);
        # iter i covers chunks j = i*NCH .. i*NCH+10, chunk ci -> kt 3ci..
        at(1, 5, pv_group(0, 1))          # kt0-2
        at(1, 6, pv_group(1, 2))          # kt3-5
        at(1, 9, pv_group(2, 4))          # kt6-11
        at(1, 10, pv_group(4, 5))         # kt12-14
        at(2, 1, pv_group(5, 6))          # kt15-17
        at(2, 2, pv_group(6, 8))          # kt18-23
        at(2, 5, pv_group(8, NCH))        # kt24-31
        at(2, 6, lambda: emit_tail(0))
        at(2, 7, pv_group(NCH, NCH + 4))
        at(2, 8, pv_group(NCH + 4, NCH + 7))
        at(2, 9, pv_group(NCH + 7, 2 * NCH))
        at(2, 10, lambda: emit_tail(1))
        at(3, 0, pv_group(2 * NCH, 2 * NCH + 3))
        at(3, 1, pv_group(2 * NCH + 3, 2 * NCH + 6))
        at(3, 2, pv_group(2 * NCH + 6, 2 * NCH + 9))
        at(3, 3, pv_group(2 * NCH + 9, 3 * NCH))
        at(3, 4, lambda: emit_tail(2))
        at(3, 5, pv_group(3 * NCH, 3 * NCH + 3))
        at(3, 6, pv_group(3 * NCH + 3, 3 * NCH + 6))
        at(3, 7, pv_group(3 * NCH + 6, 3 * NCH + 8))
        at(3, 8, pv_group(3 * NCH + 8, 3 * NCH + 9))
        at(3, 9, pv_group(3 * NCH + 9, 3 * NCH + 10))
        at(3, 10, pv_group(3 * NCH + 10, 4 * NCH))
        at(3, 10, lambda: emit_tail(3))

        # output-projection blocks: C(qb N) spread over iter N+4 (qb6 on 14);
        # by then the stage rows are written and the sync DMA queue is idle.
        C_ITER = [6, 7, 8, 9, 10, 12, 14]
        for qb in range(QB - 1):
            slots = [(C_ITER[qb], ci) for ci in range(2, NCH)]
            for op, slot in zip(c_ops(qb), slots):
                at(slot[0], slot[1], op)

        # qb7 head-0 denominators + ops0 matmuls can run during iter 15
        # (they only need iter 14's tail); the rest after the loop.
        qb7_head0 = [c_transpose(QB - 1, heads=(0,), pop=False)]
        qb7_head0 += [c_mm(QB - 1, sl, 0) for sl in range(4)]
        for op, ci in zip(qb7_head0, range(2, 7)):
            at(15, ci, op)

        # ------------------------- main loop --------------------------------
        emit_kq_proj("k", 0, 512)
        emit_kq_proj("q", 0, 512)
        emit_qk(0)
        for j in range(total):
            it_idx, qb, h, c0, n = chunk_info(j)
            ci = j - it_idx * NCH
            lg = lg_tiles.pop(j)
            ptt = ptp.tile([P, CH * 512], F16, tag="pt", name="ptt")
            if ci in DVE_CHUNKS:
                nc.vector.tensor_scalar(
                    out=ptt[:, : n * 512].bitcast(I16),
                    in0=lg[:, : n * 512],
                    scalar1=SCH_A,
                    scalar2=SCH_B,
                    op0=MULT,
                    op1=ADD,
                )
            else:
                nc.scalar.activation(
                    ptt[:, : n * 512], lg[:, : n * 512], EXP, scale=0.125
                )
            ptt_tiles[j] = ptt
            for fn in hooks.pop((it_idx, ci), []):
                fn()
            if j + 1 < total:
                emit_qk(j + 1)
            if it_idx >= 4:
                emit_pv(j)
                if ci == NCH - 1:
                    emit_tail(it_idx)

        assert not hooks, f"unconsumed hooks: {sorted(hooks)}"
        # last query block's tail: h1 denominators + second-half matmuls,
        # alternating psum banks so each matmul overlaps the previous DVE step
        c_transpose(QB - 1, heads=(1,))()
        for sl in range(4):
            pl, tg = (pbp, "pb") if sl % 2 else (mp, "mA")
            c_mm(QB - 1, sl, 1, pl, tg)()

        if dbg is not None:
            nc.sync.dma_start(dbg["kT"], kT)
            nc.sync.dma_start(dbg["qT0"], qT0)
            nc.sync.dma_start(dbg["qT1"], qT1)
            nc.sync.dma_start(dbg["vaug0"], vaug0.rearrange("p t m -> p (t m)"))
            nc.sync.dma_start(dbg["vaug1"], vaug1.rearrange("p t m -> p (t m)"))
            nc.sync.dma_start(dbg["uctx16"], uctx16)
            nc.sync.dma_start(dbg["rd"], rd.rearrange("p t m -> p (t m)"))


def build(enable_asserts=False, debug_dump=False):
    nc = bacc.Bacc(
        "TRN2",
        target_bir_lowering=False,
        debug=False,
        enable_asserts=enable_asserts,
        num_devices=N_CORES,
    )
    xq = nc.dram_tensor("xq", [S_FULL, D], F16, kind="ExternalInput").ap()
    xk = nc.dram_tensor("xk", [S_FULL, D], F16, kind="ExternalInput").ap()
    xv = nc.dram_tensor("xv", [S_FULL, D], F16, kind="ExternalInput").ap()
    wq = nc.dram_tensor("wq", [D, GD], F16, kind="ExternalInput").ap()
    wk = nc.dram_tensor("wk", [D, GD], F16, kind="ExternalInput").ap()
    wv = nc.dram_tensor("wv", [D, GD], F16, kind="ExternalInput").ap()
    wo = nc.dram_tensor("wo", [GD, D], F16, kind="ExternalInput").ap()
    bq = nc.dram_tensor("bq", [GD], F32, kind="ExternalInput").ap()
    bk = nc.dram_tensor("bk", [GD], F32, kind="ExternalInput").ap()
    out = nc.dram_tensor("out", [S_FULL, D], F16, kind="ExternalOutput").ap()
    io = (xq, xk, xv, wq, wk, wv, wo, bq, bk, out)
    dbg = None
    if debug_dump:
        dbg = {
            "kT": nc.dram_tensor("d_kT", [P, S_FULL], F16, kind="ExternalOutput").ap(),
            "qT0": nc.dram_tensor("d_qT0", [P, S_FULL], F16, kind="ExternalOutput").ap(),
            "qT1": nc.dram_tensor("d_qT1", [P, S_FULL], F16, kind="ExternalOutput").ap(),
            "vaug0": nc.dram_tensor("d_vaug0", [P, NT * P], F16, kind="ExternalOutput").ap(),
            "vaug1": nc.dram_tensor("d_vaug1", [P, NT * P], F16, kind="ExternalOutput").ap(),
            "uctx16": nc.dram_tensor("d_uctx16", [P, S_FULL], F16, kind="ExternalOutput").ap(),
            "rd": nc.dram_tensor("d_rd", [P, NT * 2], F32, kind="ExternalOutput").ap(),
        }
    with tile.TileContext(nc) as tc:
        _emit(tc, io, dbg)
    nc.compile()
    return nc


def make_in_maps(queries, keys, values, Wq, bq, Wk, bk, Wv, bv, Wo, bo):
    f16 = lambda a: np.ascontiguousarray(
        np.asarray(a, dtype=np.float32).astype(np.float16)
    )
    f32 = lambda a: np.ascontiguousarray(np.asarray(a, dtype=np.float32))
    in_maps = []
    for c in range(N_CORES):
        b, g = divmod(c, 4)
        sl = slice(g * GD, (g + 1) * GD)
        in_maps.append(
            {
                "xq": f16(queries[b]),
                "xk": f16(keys[b]),
                "xv": f16(values[b]),
                "wq": f16(np.asarray(Wq)[:, sl]),
                "wk": f16(np.asarray(Wk)[:, sl]),
                "wv": f16(np.asarray(Wv)[:, sl]),
                "wo": f16(np.asarray(Wo)[sl, :]),
                "bq": f32(np.asarray(bq)[sl]),
                "bk": f32(np.asarray(bk)[sl]),
            }
        )
    return in_maps


_NC = None
last_results = None


def kernel(queries, keys, values, Wq, bq, Wk, bk, Wv, bv, Wo, bo):
    global _NC, last_results
    if _NC is None:
        _NC = build(debug_dump=bool(int(os.environ.get("MHA_DEBUG", "0"))))
    in_maps = make_in_maps(
        queries, keys, values, Wq, bq, Wk, bk, Wv, bv, Wo, bo
    )
    res = run_bass_kernel_spmd(
        _NC,
        in_maps,
        core_ids=list(range(N_CORES)),
        trace=bool(int(os.environ.get("MHA_TRACE", "0"))),
    )
    last_results = res
    outs = [
        np.asarray(res.results[c]["out"], dtype=np.float32)
        for c in range(N_CORES)
    ]
    full = np.empty((B_FULL, S_FULL, D), dtype=np.float32)
    # attention rows sum to 1, so the value bias contributes bv @ Wo to
    # every output row; fold it into the host-side constant with bo.
    bv16 = np.asarray(bv, np.float32).astype(np.float16).astype(np.float32)
    wo16 = np.asarray(Wo, np.float32).astype(np.float16).astype(np.float32)
    const = np.asarray(bo, dtype=np.float32) + bv16 @ wo16
    for b in range(B_FULL):
        full[b] = outs[4 * b] + outs[4 * b + 1] + outs[4 * b + 2] + outs[4 * b + 3]
        full[b] += const
    return full

